# revision 39
# baseline (speedup 1.0000x reference)
"""CASCADES adapter (moe_routing) Trainium2 kernel.

Reference math:
    centroid = 0.7*x[:,-1,:] + 0.3*mean_s(x)           [B, IN]
    w        = softmax(cos(centroid, core_keys)/TEMP)  [B, K]
    Lam[b]   = sum_k w[b,k] * core_pool[k]             [B, R, R]
    out      = gate * x @ V^T @ Lam^T @ U^T            [B, S, OUT]
gate is a scalar depending only on U, V, gate_w, gate_b (host-computed).

Restructuring:
    out[b] = xV[b] @ UL[b]^T,   xV = x @ V^T (rank R=8),
    UL[b]  = gate * U @ Lam[b]  [OUT, R]  (tiny, host-computed)
Routing needs only per-batch column sums of x (device-computed in stage 1),
x[:,-1,:] and tiny tensors (host).

Sharding: 8 cores, core c owns batch c//2, S rows [(c%2)*2048, (c%2+1)*2048).

Precision: the accuracy budget (rel_err < 2e-2) is far looser than fp32, so
both big transfers run in fp16 (~1.5e-4 rms rounding): stage 1 reads each x
shard once as fp16 (16 MB/core), stage 2 writes each output shard once as
fp16 (16 MB/core, host upcasts to fp32). Column sums accumulate in fp32 via
ScalarE/VectorE accum_out, so routing keeps ~1e-4 accuracy. End-to-end
rel_err ~5e-4.
"""

import os
from contextlib import ExitStack

import numpy as np

import concourse.tile as tile
from concourse import bacc, mybir
from concourse.bass_utils import run_bass_kernel_spmd

FP = mybir.dt.float32
F16 = mybir.dt.float16

B, S, IN, OUT, R, K = 4, 4096, 4096, 4096, 8, 4
NCORES = 8
SSH = S // 2          # 2048: per-core S shard
NI_CH = IN // 128     # 32 contraction chunks
EPS = 1e-8
TEMP = 0.05

# Populated on every kernel() call when KERNEL_TRACE=1.
LAST_STATS: dict = {}

_prog_cache: dict = {}


def build_stage1():
    """Per core:
      xv[r, s] = sum_i V[r,i] * xT[i, s]    (fp16 PE, fp32 PSUM accumulate)
      cs[i-part, ic]: free-axis sums of each x chunk (fp32 accum_out, split
      ScalarE/VectorE; host reassembles -> column sums of x)
    Input xt [IN, SSH] fp16 (x shard transposed).
    Input vt [128, NI_CH*R] fp16: V^T chunk-major.
    """
    nc = bacc.Bacc("TRN2", target_bir_lowering=False, debug=False, num_devices=NCORES)
    xt_d = nc.dram_tensor("xt", [IN, SSH], F16, kind="ExternalInput").ap()
    vt_d = nc.dram_tensor("vt", [128, NI_CH * R], F16, kind="ExternalInput").ap()
    xv_d = nc.dram_tensor("xv", [R, SSH], F16, kind="ExternalOutput").ap()
    cs_d = nc.dram_tensor("cs", [128, 2 * NI_CH], FP, kind="ExternalOutput").ap()

    with tile.TileContext(nc) as tc:
        with ExitStack() as ctx:
            xin = ctx.enter_context(tc.tile_pool(name="xin", bufs=14))
            scr = ctx.enter_context(tc.tile_pool(name="scr", bufs=2))
            scr2 = ctx.enter_context(tc.tile_pool(name="scr2", bufs=2))
            small = ctx.enter_context(tc.tile_pool(name="small", bufs=1))
            psum = ctx.enter_context(tc.tile_pool(name="psum", bufs=1, space="PSUM"))

            # load V on the ScalarE HWDGE queue so x-chunk loads start
            # immediately on the sync queue
            v_sb = small.tile([128, NI_CH * R], F16)
            nc.scalar.dma_start(v_sb[:], vt_d[:])
            acc = small.tile([128, 2 * NI_CH], FP)  # 2 partial sums per chunk
            # s-slice sb accumulates at partitions 32*sb..+8, bank sb
            # (PE column tiling: 4 concurrent 128x32 tiles; one accumulation
            # group per PSUM bank - groups are bank-granular).
            xvp = psum.tile([128, 4 * 512], FP)

            NSB = SSH // 512  # 4 rhs slices per chunk
            SPL = 704         # colsum split: ScalarE [0:704), VectorE [704:)
            for ic in range(NI_CH):
                last = ic == NI_CH - 1
                xt = xin.tile([128, SSH], F16)
                if not last:
                    nc.sync.dma_start(xt[:], xt_d[ic * 128:(ic + 1) * 128, :])
                else:
                    # last chunk loads in 4 slices so its colsum + matmuls
                    # drain while later slices still stream (shorter tail)
                    for sb in range(NSB):
                        nc.sync.dma_start(
                            xt[:, sb * 512:(sb + 1) * 512],
                            xt_d[ic * 128:(ic + 1) * 128,
                                 sb * 512:(sb + 1) * 512])
                # column sums (free-axis accumulate), split across engines
                sc_t = scr.tile([128, SPL], F16)
                nc.scalar.activation(
                    sc_t[:], xt[:, 0:SPL], mybir.ActivationFunctionType.Copy,
                    accum_out=acc[:, ic:ic + 1])
                sc_t2 = scr2.tile([128, SSH - SPL], F16)
                nc.vector.tensor_scalar(
                    sc_t2[:], xt[:, SPL:SSH], 1.0, None, mybir.AluOpType.mult,
                    mybir.AluOpType.add,
                    accum_out=acc[:, NI_CH + ic:NI_CH + ic + 1])
                vh = v_sb[:, ic * R:(ic + 1) * R]
                for sb in range(NSB):
                    nc.tensor.matmul(
                        xvp[32 * sb:32 * sb + R, sb * 512:(sb + 1) * 512],
                        vh,
                        xt[:, sb * 512:(sb + 1) * 512],
                        start=(ic == 0),
                        stop=(ic == NI_CH - 1),
                        tile_position=(0, 32 * sb),
                    )

            # per-engine staging tiles (a writer pair sharing one tile gets
            # serialized by the scheduler) and per-engine stores. DRAM column
            # order becomes [sb0, sb2 | sb1, sb3]; host un-permutes.
            xv_dve = small.tile([R, 1024], F16)
            xv_act = small.tile([R, 1024], F16)
            for sb in range(NSB):
                src = xvp[32 * sb:32 * sb + R, sb * 512:(sb + 1) * 512]
                half = (sb // 2) * 512
                if sb % 2 == 0:
                    nc.vector.tensor_copy(xv_dve[:, half:half + 512], src)
                else:
                    nc.scalar.copy(xv_act[:, half:half + 512], src)
            # cs first: its deps (colsums) resolve before the xv copies, so
            # its dispatch+HWDGE overlaps the copy chain
            nc.sync.dma_start(cs_d[:], acc[:])
            nc.sync.dma_start(xv_d[:, 0:1024], xv_dve[:])
            nc.sync.dma_start(xv_d[:, 1024:2048], xv_act[:])

    nc.compile()
    return nc


def build_stage2():
    """Per core: out[s, o] = sum_r xv[r, s] * ulT[r, o]  (fp16, fp16 store).

    K=8 contraction: weights are tiny (8x128), so LDWEIGHTS is negligible
    next to the 512-col streams; no row-group rotation needed. The critical
    resource is the store DMA (1456 ns per 512 KB), so the loop keeps 4
    quarter-size PSUM tiles in flight, each evacuated whole by one engine
    (DVE/ACT alternating) so the two copies of each store run in parallel.
    """
    nc = bacc.Bacc("TRN2", target_bir_lowering=False, debug=False, num_devices=NCORES)
    xv_d = nc.dram_tensor("xvh", [R, SSH], F16, kind="ExternalInput").ap()
    ul_d = nc.dram_tensor("ulh", [R, OUT], F16, kind="ExternalInput").ap()
    out_d = nc.dram_tensor("out", [SSH, OUT], F16, kind="ExternalOutput").ap()

    with tile.TileContext(nc) as tc:
        with ExitStack() as ctx:
            small = ctx.enter_context(tc.tile_pool(name="small", bufs=1))
            ostage = ctx.enter_context(tc.tile_pool(name="ostage", bufs=8))
            psum = ctx.enter_context(tc.tile_pool(name="psum", bufs=4, space="PSUM"))

            xv_sb = small.tile([R, SSH], F16)
            nc.sync.dma_start(xv_sb[:], xv_d[:])
            ul_sb = small.tile([R, OUT], F16)
            nc.sync.dma_start(ul_sb[:], ul_d[:])
            for sc in range(SSH // 128):       # 16 s-chunks
                nst = 2                        # 2 stores of [128, 2048] each
                qpst = 4 // nst                # psum quarters per store
                for oh in range(nst):
                    ot = ostage.tile([128, 1024 * qpst], F16)
                    for q in range(qpst):
                        op = psum.tile([128, 1024], FP)  # 2 banks
                        for ob in range(2):
                            xh = xv_sb[:, sc * 128:(sc + 1) * 128]
                            o0 = (oh * qpst + q) * 1024 + ob * 512
                            uh = ul_sb[:, o0:o0 + 512]
                            nc.tensor.matmul(
                                op[:, ob * 512:(ob + 1) * 512], xh, uh,
                                start=True, stop=True)
                        # whole-quarter PSUM evacuation (fp32->fp16 cast);
                        # alternate engines so adjacent quarters copy in
                        # parallel
                        dst = ot[:, q * 1024:(q + 1) * 1024]
                        if (oh * qpst + q) % 2 == 0:
                            nc.vector.tensor_copy(dst, op[:])
                        else:
                            nc.scalar.copy(dst, op[:])
                    o0 = oh * qpst * 1024
                    nc.sync.dma_start(
                        out_d[sc * 128:(sc + 1) * 128,
                              o0:o0 + 1024 * qpst], ot[:])

    nc.compile()
    return nc


def _get_prog(name, builder):
    if name not in _prog_cache:
        _prog_cache[name] = builder()
    return _prog_cache[name]


def _run(nc, in_maps, core_ids, trace):
    """run_bass_kernel_spmd, falling back to trace=False when the NTFF
    profile hook is unavailable in this environment."""
    try:
        return run_bass_kernel_spmd(nc, in_maps, core_ids, trace=trace)
    except (ImportError, ModuleNotFoundError):
        if not trace:
            raise
        return run_bass_kernel_spmd(nc, in_maps, core_ids, trace=False)


def _routing_host(colsum, x_last, V_shared, U_shared, core_pool, core_keys,
                  gate_w, gate_b):
    """All tiny routing math in float64. colsum: [B, IN] sums over S.
    Returns UL[b] = gate * U @ Lam[b]  [B, OUT, R]."""
    m = colsum / S
    xl = x_last.astype(np.float64)
    centroid = 0.7 * xl + 0.3 * m
    cn = centroid / np.maximum(
        np.linalg.norm(centroid, axis=-1, keepdims=True), EPS)
    kn = core_keys.astype(np.float64)
    kn = kn / np.maximum(np.linalg.norm(kn, axis=-1, keepdims=True), EPS)
    sim = cn @ kn.T
    z = sim / TEMP
    z = z - z.max(axis=-1, keepdims=True)
    w = np.exp(z)
    w = w / w.sum(axis=-1, keepdims=True)
    Lam = np.einsum("bk,kij->bij", w, core_pool.astype(np.float64))
    gate_in = np.concatenate([
        U_shared.astype(np.float64).mean(axis=0),
        V_shared.astype(np.float64).mean(axis=1)])
    gate = 1.0 / (1.0 + np.exp(
        -(gate_w.astype(np.float64) @ gate_in + gate_b.astype(np.float64))))
    UL = gate[0] * np.einsum("oj,bjr->bor", U_shared.astype(np.float64), Lam)
    return UL


def kernel(x, V_shared, U_shared, core_pool, core_keys, gate_w, gate_b):
    trace = os.environ.get("KERNEL_TRACE", "") == "1"
    core_ids = list(range(NCORES))

    x = np.asarray(x, dtype=np.float32)
    V_shared = np.asarray(V_shared, dtype=np.float32)
    U_shared = np.asarray(U_shared, dtype=np.float32)
    core_pool = np.asarray(core_pool, dtype=np.float32)
    core_keys = np.asarray(core_keys, dtype=np.float32)
    gate_w = np.asarray(gate_w, dtype=np.float32)
    gate_b = np.asarray(gate_b, dtype=np.float32)

    # ---- host prep: per-core transposed fp16 shards
    xts = []
    for c in range(NCORES):
        xs = x[c // 2, (c % 2) * SSH:(c % 2 + 1) * SSH, :].T
        xts.append(np.ascontiguousarray(xs.astype(np.float16)))  # [IN, SSH]

    # V^T chunk-major: [R, IN] -> [128, NI_CH*R]
    vt = np.ascontiguousarray(
        V_shared.T.reshape(NI_CH, 128, R).transpose(1, 0, 2)
        .reshape(128, NI_CH * R).astype(np.float16))

    # ---- stage 1 on device
    nc1 = _get_prog("s1", build_stage1)
    r1 = _run(nc1, [{"xt": xts[c], "vt": vt} for c in core_ids], core_ids, trace)
    # xv DRAM column order is [sb0, sb2 | sb1, sb3] (per-engine stores);
    # un-permute back to natural s order
    xvs = []
    for c in core_ids:
        xvr = r1.results[c]["xv"]  # [R, SSH] fp16, permuted
        xvn = np.empty_like(xvr)
        xvn[:, 0:512] = xvr[:, 0:512]        # sb0 (dve half, first)
        xvn[:, 1024:1536] = xvr[:, 512:1024]  # sb2 (dve half, second)
        xvn[:, 512:1024] = xvr[:, 1024:1536]  # sb1 (act half, first)
        xvn[:, 1536:2048] = xvr[:, 1536:2048]  # sb3 (act half, second)
        xvs.append(xvn)
    css = [r1.results[c]["cs"] for c in core_ids]  # [128, 2*NI_CH] fp32

    # ---- routing on host (tiny); cs = [scalar part | vector part], add both
    def core_colsum(csm):  # [128, 2*NI_CH] -> [IN]
        m = csm.astype(np.float64)
        return (m[:, :NI_CH] + m[:, NI_CH:]).T.reshape(IN)

    colsum = np.stack([
        core_colsum(css[2 * b]) + core_colsum(css[2 * b + 1]) for b in range(B)
    ])
    UL = _routing_host(colsum, x[:, -1, :], V_shared, U_shared, core_pool,
                       core_keys, gate_w, gate_b)

    # ---- stage 2 inputs (fp16)
    xvhs = [np.ascontiguousarray(xvs[c].astype(np.float16)) for c in range(NCORES)]
    ulhs = []
    for c in range(NCORES):
        ulhs.append(np.ascontiguousarray(
            UL[c // 2].T.astype(np.float32).astype(np.float16)))  # [R, OUT]

    nc2 = _get_prog("s2", build_stage2)
    r2 = _run(nc2, [{"xvh": xvhs[c], "ulh": ulhs[c]} for c in core_ids],
              core_ids, trace)
    outs = [r2.results[c]["out"].astype(np.float32) for c in core_ids]

    if trace:
        LAST_STATS.clear()
        LAST_STATS["stage1_ns"] = r1.exec_time_ns
        LAST_STATS["stage2_ns"] = r2.exec_time_ns
        LAST_STATS["total_ns"] = (
            (r1.exec_time_ns or 0) + (r2.exec_time_ns or 0)
            if (r1.exec_time_ns or r2.exec_time_ns) else None)

    return np.stack(
        [np.concatenate([outs[2 * b], outs[2 * b + 1]], axis=0) for b in range(B)]
    )
